# revision 68
# baseline (speedup 1.0000x reference)
"""Causal self-attention (B=4, T=2048, D=1024, H=16, hd=64) on 8 TRN2 NeuronCores.

Sharding: core c handles batch b = c % 4 and head-half = c // 4 (8 heads each).
Each core computes, for its (batch, 8 heads):
    qkv projection -> causal attention -> partial output projection (yT).
Host gathers: y[b] = (yT[core b] + yT[core b+4]).T + b_proj.

Device design (per core), v9 (fp16 + fp8 q/k projection):
  - PSUM fp32; attention fp16/bf16; q/k projection in fp8e4m3 DoubleRow
    (2 fp8 weights per PE cell -> 256-row contraction per matmul, ~2x the
    fp16 rate; rel err 1.5e-2 vs the 2e-2 gate, deterministic)
  - transposed layouts: xT [D, T], qT/kT [2heads x 64, pair, T], V [t, kt, 8h*64]
  - S^T [k, q] tiles: two heads row-packed at tile_position (0,0)/(64,0)
  - exp on ScalarE (scale fused); causal mask via gpsimd affine_select on the
    128-wide diagonal block only
  - PV col-packed: head A -> psum rows 0:64, head B -> rows 64:128; both
    matmuls run concurrently (one 512-cyc slot)
  - softmax denominator: DVE accumulates e tiles into a per-pair partial
    (bf16 2x DVE); one col-packed pair of ones[128,64] matmuls reduces AND
    broadcasts the denominators into a [128,512] psum; DVE reciprocal -> bc;
    the PV psum drain is fused with the normalize multiply
  - attention k-tiles processed in BATCHES of 2; QK,QK back-to-back, PV
    batches deferred and popped two-at-a-time on odd batches (4 PV slots
    back-to-back).  Same-type consecutive PE slots cost ~5-30ns vs
    ~110-200ns for type changes (measured), so batching cuts most of the
    kt-loop transition overhead.
  - host pre-arranges x/wqk/wv/wp in partition-major DRAM layouts so every
    initial DMA moves 8KB-contiguous per partition; startup DMAs ride two
    queues in strict priority order (sync: xc0_8, wqk8, xc0, xc1_8, xc1, wp;
    gpsimd: bqk, bv, wv) - the aggregate HBM rate is shared at packet
    granularity, so queue-FIFO order is the only way to prioritize
  - steady-state DMA issues stay off the Scalar queue (exp cadence)
  - final projection chunk split: 3-pair partial matmuls run as filler
    inside the last attention pair and ship via yT2 (host adds), leaving
    only 8 single matmuls at the end - drained into 8 psum banks at once,
    copied out on DVE+ScalarE in parallel, shipped with 2 batched DMAs
  - emission interleaves p1/proj matmul "filler" units into the attention
    batch loop (engines are FIFO; emission order = execution order)
  - PE warmup matmuls cover the startup weight-load window and keep the HAM
    clock gate open
"""

import sys
from collections import deque
from contextlib import ExitStack

import numpy as np

sys.path.insert(0, "/opt/trn_rl_repo")

import concourse.bass as bass  # noqa: E402
import concourse.tile as tile  # noqa: E402
from concourse import bacc, mybir  # noqa: E402

f16 = mybir.dt.float16
bf16 = mybir.dt.bfloat16
f32 = mybir.dt.float32
f8 = mybir.dt.float8e4
DR = mybir.MatmulPerfMode.DoubleRow
EXP = mybir.ActivationFunctionType.Exp
ADD = mybir.AluOpType.add
MULT = mybir.AluOpType.mult

B, T, D = 4, 2048, 1024
H, HD = 16, 64
HPC = 8            # heads per core
NP = 4             # head pairs per core
NCORES = 8
TCH = 512          # p1 t-chunk width == attention q-chunk width
NCH = T // TCH     # 4
QCH = 512
NKT = T // 128     # 16 k-tiles
WARMUP = 160


def build_program():
    nc = bacc.Bacc("TRN2", target_bir_lowering=False, debug=False)

    # partition-major DRAM layouts: per-partition data is 8KB-contiguous
    xr = nc.dram_tensor("xr", [128, NCH, 8, TCH], f16, kind="ExternalInput").ap()
    # fp8 copies feeding the DoubleRow q/k projection matmuls
    xr8 = nc.dram_tensor("xr8", [128, NCH, 8, TCH], f8, kind="ExternalInput").ap()
    wqk8 = nc.dram_tensor("wqk8", [128, 8, 8, 128], f8, kind="ExternalInput").ap()
    wv = nc.dram_tensor("wv", [128, 8, HPC * HD], f16, kind="ExternalInput").ap()
    wp = nc.dram_tensor("wp", [128, NP, D], f16, kind="ExternalInput").ap()
    bqk = nc.dram_tensor("bqk", [128, 8], f32, kind="ExternalInput").ap()
    bv = nc.dram_tensor("bv", [128, HPC * HD], f16, kind="ExternalInput").ap()
    yT = nc.dram_tensor("yT", [D, T], f16, kind="ExternalOutput").ap()
    # 3-pair partial of the final projection chunk (host adds yT2 onto
    # yT[:, 1536:2048]); lets that work leave the device before the last
    # attention pair finishes
    yT2 = nc.dram_tensor("yT2", [D, QCH], f16, kind="ExternalOutput").ap()

    with tile.TileContext(nc) as tc:
        with ExitStack() as ctx:
            _build(ctx, tc, xr, xr8, wqk8, wv, wp, bqk, bv, yT, yT2)
    nc.compile()
    return nc


def _build(ctx, tc, xr, xr8, wqk8, wv, wp, bqk, bv, yT, yT2):
    nc = tc.nc

    persist = ctx.enter_context(tc.tile_pool(name="persist", bufs=1))
    w_pool = ctx.enter_context(tc.tile_pool(name="w_pool", bufs=1))
    xc_pool = ctx.enter_context(tc.tile_pool(name="xc_pool", bufs=2))
    e_pool = ctx.enter_context(tc.tile_pool(name="e_pool", bufs=14))
    pa_pool = ctx.enter_context(tc.tile_pool(name="pa_pool", bufs=3))
    pin_pool = ctx.enter_context(tc.tile_pool(name="pin_pool", bufs=4))
    bc_pool = ctx.enter_context(tc.tile_pool(name="bc_pool", bufs=2))
    ysb_pool = ctx.enter_context(tc.tile_pool(name="ysb_pool", bufs=4))
    small = ctx.enter_context(tc.tile_pool(name="small", bufs=1))

    p1ps = ctx.enter_context(tc.tile_pool(name="p1ps", bufs=2, space="PSUM"))
    s_ps = ctx.enter_context(tc.tile_pool(name="s_ps", bufs=2, space="PSUM"))
    pv_ps = ctx.enter_context(tc.tile_pool(name="pv_ps", bufs=2, space="PSUM"))

    # ---- persistent tensors ----
    qT = persist.tile([128, NP, T], f16, tag="qT")      # [2 heads x 64 dims, pair, t]
    kT = persist.tile([128, NP, T], f16, tag="kT")
    V = persist.tile([128, NKT, HPC * HD], bf16, tag="V")  # [t in tile, k-tile, h*64+d]

    ones_sb = small.tile([128, 64], bf16, tag="ones_sb", bufs=1)
    nc.vector.memset(ones_sb, 1.0)

    # ---- initial DMAs ----
    # Per-queue FIFO order = priority order.  The aggregate HBM rate is shared
    # at packet granularity across queues, so the critical first-wave items
    # (xc0, wqk mt0-1) are FIRST on their queues and nothing else competes.
    def load_xc(c, eng=None):
        xc = xc_pool.tile([128, 8, TCH], f16, name=f"xc{c}", tag="xc")
        (eng or nc.sync).dma_start(out=xc, in_=xr[:, c])
        return xc

    def load_xc8(c, eng=None):
        xc8 = xc_pool.tile([128, 8, TCH], f8, name=f"xc8_{c}", tag="xc8")
        (eng or nc.sync).dma_start(out=xc8, in_=xr8[:, c])
        return xc8

    # Two-queue strict-priority startup: everything critical goes on sync in
    # FIFO order (xc0_8 -> wqk8 -> xc0 -> xc1_8 -> xc1) so the first-wave
    # bytes are not diluted by lower-priority transfers; bulky non-critical
    # loads ride gpsimd.  Scalar issues nothing - it stays clean for exp.
    # wqk8 is stored in permuted mt order [0,4,1,5,2,6,3,7] so the first
    # 128KB slice carries exactly the q+k units of attention pair 0
    # one fast queue, strict priority: FIFO order on sync == consumption
    # order of the prologue; parallel queues only dilute the critical bytes
    xc0_8 = load_xc8(0)
    wqk_sb = w_pool.tile([128, 8, 8, 128], f8, tag="wqk_sb")
    nc.sync.dma_start(out=wqk_sb[:, 0:2], in_=wqk8[:, 0:2])
    bqk_sb = small.tile([128, 8], f32, tag="bqk_sb", bufs=1)
    nc.gpsimd.dma_start(out=bqk_sb, in_=bqk)
    nc.sync.dma_start(out=wqk_sb[:, 2:8], in_=wqk8[:, 2:8])
    bv_sb = small.tile([128, HPC * HD], f16, tag="bv_sb", bufs=1)
    nc.gpsimd.dma_start(out=bv_sb, in_=bv)
    wv_sb = w_pool.tile([128, 8, HPC * HD], f16, tag="wv_sb")
    nc.sync.dma_start(out=wv_sb, in_=wv)
    # xc0 in halves: the first v-unit's dt0-3 matmuls only need the first
    # half, which lands ~1us earlier than the whole chunk would
    xc0 = xc_pool.tile([128, 8, TCH], f16, name="xc0", tag="xc")
    nc.sync.dma_start(out=xc0[:, 0:4], in_=xr[:, 0, 0:4])
    nc.sync.dma_start(out=xc0[:, 4:8], in_=xr[:, 0, 4:8])
    xc1_8 = load_xc8(1)
    xc1 = load_xc(1)
    wp_sb = w_pool.tile([128, NP, D], f16, tag="wp_sb")
    nc.sync.dma_start(out=wp_sb, in_=wp)

    # stored slot for logical mt under the [0,4,1,5,2,6,3,7] permutation
    SLOT = {mt: (mt % 4) * 2 + (mt // 4) for mt in range(8)}

    zreg = nc.gpsimd.to_reg(0.0)

    # ---- PE warmup: keep the PE busy (and the HAM gate open) during the
    # initial weight DMA; writes a never-read scratch psum ----
    scr = p1ps.tile([128, TCH], f32, name="warm", tag="p1")
    for i in range(WARMUP):
        nc.tensor.matmul(scr[0:64, 0:64], ones_sb, ones_sb, start=True, stop=True)

    # ================= phase 1 units (qkv projection) =================
    # q/k projection in fp8 DoubleRow: 2 fp8 weights per PE cell -> each
    # matmul contracts 256 rows, halving the PE time of these units
    def p1_qk_unit(c, mt, xc8):
        qk_ps = p1ps.tile([128, TCH], f32, name=f"qk{c}_{mt}", tag="p1")
        for g in range(4):
            nc.tensor.matmul(qk_ps, wqk_sb[:, SLOT[mt], 2 * g:2 * g + 2, :],
                             xc8[:, 2 * g:2 * g + 2, :],
                             start=(g == 0), stop=(g == 3), perf_mode=DR)
        dest = qT if mt < 4 else kT
        nc.vector.tensor_scalar_add(
            dest[:, mt % 4, c * TCH:(c + 1) * TCH], qk_ps, bqk_sb[:, mt:mt + 1])

    def p1_v_unit(c, tt, xc):
        v_ps = p1ps.tile([128, 512], f32, name=f"v{c}_{tt}", tag="p1")
        for dt in range(8):
            nc.tensor.matmul(v_ps, xc[:, dt, tt * 128:(tt + 1) * 128],
                             wv_sb[:, dt, :], start=(dt == 0), stop=(dt == 7))
        nc.vector.tensor_tensor(out=V[:, c * 4 + tt, :], in0=v_ps, in1=bv_sb, op=ADD)

    def p1_units(c, xc, xc8):
        units = [(lambda c=c, mt=mt, xc8=xc8: p1_qk_unit(c, mt, xc8))
                 for mt in range(8)]
        units += [(lambda c=c, tt=tt, xc=xc: p1_v_unit(c, tt, xc)) for tt in range(4)]
        return units

    # ================= phase 3 unit (output projection) =================
    dmae_rot = [nc.sync, nc.gpsimd, nc.scalar, nc.sync]

    def proj_unit(j, pin, mt):
        y_ps = p1ps.tile([128, QCH], f32, name=f"y{j}_{mt}", tag="p1")
        for p in range(NP):
            nc.tensor.matmul(y_ps, wp_sb[:, p, mt * 128:(mt + 1) * 128],
                             pin[:, p, :], start=(p == 0), stop=(p == NP - 1))
        y_sb = ysb_pool.tile([128, QCH], f16, name=f"ysb{j}_{mt}", tag="ysb")
        if j == 3:
            # final chunk: alternate DVE/ScalarE drains; rotate DMA queues so
            # the tail does not serialize on one queue
            if mt % 2 == 0:
                nc.vector.tensor_copy(out=y_sb, in_=y_ps)
            else:
                nc.scalar.copy(out=y_sb, in_=y_ps)
            dmae_rot[mt % 4].dma_start(
                out=yT[mt * 128:(mt + 1) * 128, j * QCH:(j + 1) * QCH], in_=y_sb)
        else:
            nc.vector.tensor_copy(out=y_sb, in_=y_ps)
            nc.sync.dma_start(
                out=yT[mt * 128:(mt + 1) * 128, j * QCH:(j + 1) * QCH], in_=y_sb)

    def interleave(a, b):
        out = []
        la, lb = list(a), list(b)
        n = max(len(la), len(lb))
        for i in range(n):
            if i < len(la):
                out.append(la[i])
            if i < len(lb):
                out.append(lb[i])
        return out

    def proj_units(j, pin):
        return [(lambda j=j, pin=pin, mt=mt: proj_unit(j, pin, mt)) for mt in range(8)]

    # final chunk's projection split: a 3-pair partial that ships via yT2
    # while the last attention pair runs (host adds it), plus a 1-matmul
    # finisher per tile, so only ~2us of PE+drain work follows the last
    # attention slot
    def proj3a_unit(pin, mt):
        y_ps = p1ps.tile([128, QCH], f32, name=f"y3a_{mt}", tag="p1")
        for p in range(3):
            nc.tensor.matmul(y_ps, wp_sb[:, p, mt * 128:(mt + 1) * 128],
                             pin[:, p, :], start=(p == 0), stop=(p == 2))
        yp = ysb_pool.tile([128, QCH], f16, name=f"y3p{mt}", tag="ysb")
        nc.vector.tensor_copy(out=yp, in_=y_ps)
        [nc.sync, nc.gpsimd][mt % 2].dma_start(
            out=yT2[mt * 128:(mt + 1) * 128, :], in_=yp)

    y3b_all = persist.tile([128, 8, QCH], f16, tag="y3b")

    def proj3_final(pin):
        # by now all attention psum pools are free: run all 8 finisher
        # matmuls back-to-back into 8 banks, drain on DVE+ScalarE in
        # parallel, ship with two batched DMAs
        pss = []
        s_tiles = []
        s_full = None
        for mt in range(8):
            if mt < 2:
                ps = p1ps.tile([128, QCH], f32, name=f"y3b{mt}", tag="p1")
            elif mt < 4:
                ps = pv_ps.tile([128, QCH], f32, name=f"y3b{mt}", tag="pv")
            else:
                if mt % 2 == 0:
                    s_full = s_ps.tile([128, 2 * QCH], f32,
                                       name=f"y3s{mt}", tag="s")
                    s_tiles.append(s_full)
                ps = s_full[:, 0:QCH] if mt % 2 == 0 else s_full[:, QCH:2 * QCH]
            nc.tensor.matmul(ps, wp_sb[:, 3, mt * 128:(mt + 1) * 128],
                             pin[:, 3, :], start=True, stop=True)
            pss.append(ps)
        # alternate DVE/ScalarE drains; ship each mt-pair the moment both of
        # its drains are done (the last DMA's HBM receipt is the tail's
        # critical path, so keep it small and early)
        yv = yT.rearrange("(m p) t -> p m t", p=128)
        for mt in range(8):
            if mt % 2 == 0:
                nc.vector.tensor_copy(out=y3b_all[:, mt, :], in_=pss[mt])
            else:
                nc.scalar.copy(out=y3b_all[:, mt, :], in_=pss[mt])
                [nc.sync, nc.gpsimd][(mt // 2) % 2].dma_start(
                    out=yv[:, mt - 1:mt + 1, 3 * QCH:4 * QCH],
                    in_=y3b_all[:, mt - 1:mt + 1, :])

    # ================= phase 2: attention =================
    # deferred: PE-op closures executed with a lag of 2 kt-batches so the PE
    # (FIFO) never queues a PV matmul whose exp/mask input isn't ready yet.
    deferred = deque()
    fillers = deque()
    fill_state = {"acc": 0.0}

    def attn_pair(j, p, pin, rate):
        nkt = 4 * (j + 1)
        q0 = j * QCH
        partial = pa_pool.tile([128, 2 * QCH], bf16, name=f"pa{j}_{p}", tag="pa")
        pv = pv_ps.tile([128, QCH], f32, name=f"pv{j}_{p}", tag="pv")
        for bt in range(nkt // 2):
            kts = (2 * bt, 2 * bt + 1)
            # 1) QK slots back-to-back (same-type transitions are cheap)
            svs = []
            for kt in kts:
                o = max(0, kt - 4 * j)
                c0 = 128 * o
                s = s_ps.tile([128, 2 * QCH], f32, name=f"s{j}_{p}_{kt}", tag="s")
                nc.tensor.matmul(
                    s[:, c0:QCH], kT[0:64, p, kt * 128:(kt + 1) * 128],
                    qT[0:64, p, q0 + c0:q0 + QCH],
                    start=True, stop=True, tile_position=(0, 0))
                nc.tensor.matmul(
                    s[:, QCH + c0:2 * QCH], kT[64:128, p, kt * 128:(kt + 1) * 128],
                    qT[64:128, p, q0 + c0:q0 + QCH],
                    start=True, stop=True, tile_position=(64, 0))
                svs.append((kt, o, c0, s))
            # 2) on odd batches, pop two deferred PV batches (plus any fins)
            # so 4 PV slots run back-to-back - same-type PE transitions are
            # ~30x cheaper than type changes.  In the chunk-2/3 region keep
            # one extra batch deferred: the exp->mask->PV chain on diagonal
            # tiles was the source of recurring ~0.5us PE gaps there.
            if bt % 2 == 1:
                thr = 3 if j >= 2 else 1
                while len(deferred) > thr:
                    deferred.popleft()()
            # 3) exp + mask + denominator partials (Scalar/GpSimd/DVE)
            es = []
            for kt, o, c0, s in svs:
                e = e_pool.tile([128, 2 * QCH], bf16, name=f"e{j}_{p}_{kt}", tag="e")
                if o == 0:
                    nc.scalar.activation(e, s, EXP, scale=0.125)
                else:
                    sv = s.rearrange("p (h q) -> p h q", h=2)[:, :, c0:QCH]
                    ev = e.rearrange("p (h q) -> p h q", h=2)[:, :, c0:QCH]
                    nc.scalar.activation(ev, sv, EXP, scale=0.125)
                if kt >= 4 * j:
                    # mask only the 128-wide diagonal block: keep col >= k
                    ev2 = e.rearrange("p (h q) -> p h q", h=2)[:, :, c0:c0 + 128]
                    nc.gpsimd.affine_select(
                        ev2, ev2, pattern=[[0, 2], [1, 128]],
                        compare_op=mybir.AluOpType.is_ge, fill=zreg,
                        base=0, channel_multiplier=-1)
                if kt == 0:
                    nc.vector.tensor_copy(out=partial, in_=e)
                elif o == 0:
                    nc.vector.tensor_tensor(out=partial, in0=partial, in1=e, op=ADD)
                else:
                    pview = partial.rearrange("p (h q) -> p h q", h=2)[:, :, c0:QCH]
                    eview = e.rearrange("p (h q) -> p h q", h=2)[:, :, c0:QCH]
                    nc.vector.tensor_tensor(out=pview, in0=pview, in1=eview, op=ADD)
                es.append((kt, c0, e))

            # 4) defer this batch's PV pair-slots
            def emit_pv_batch(es=es, pv=pv, nkt=nkt, p=p):
                for kt, c0, e in es:
                    last = (kt == nkt - 1)
                    nc.tensor.matmul(
                        pv[0:64, c0:QCH], V[:, kt, (2 * p) * 64:(2 * p + 1) * 64],
                        e[:, c0:QCH], start=(kt == 0), stop=last)
                    nc.tensor.matmul(
                        pv[64:128, c0:QCH], V[:, kt, (2 * p + 1) * 64:(2 * p + 2) * 64],
                        e[:, QCH + c0:2 * QCH], start=(kt == 0), stop=last)

            deferred.append(emit_pv_batch)
            # 5) fillers - popped per batch: the steady drip spaces QK
            # batches at the exp cadence (clumping them stalls the next QK
            # batch on s_ps/exp recycling - measured +44us)
            fill_state["acc"] += rate
            while fillers and fill_state["acc"] >= 1.0:
                fillers.popleft()()
                fill_state["acc"] -= 1.0

        def emit_fin(j=j, p=p, partial=partial, pv=pv, pin=pin):
            # ones[128,64] matmuls: reduce the partial over k AND broadcast
            # the per-head denominators into all 64 rows of a psum bank
            dn = p1ps.tile([128, QCH], f32, name=f"dn{j}_{p}", tag="p1")
            nc.tensor.matmul(dn[0:64, :], ones_sb, partial[:, 0:QCH],
                             start=True, stop=True)
            nc.tensor.matmul(dn[64:128, :], ones_sb, partial[:, QCH:2 * QCH],
                             start=True, stop=True)
            bc = bc_pool.tile([128, QCH], f32, name=f"bc{j}_{p}", tag="bc")
            nc.vector.reciprocal_approx_fast(out=bc, in_=dn)
            # fused PV-psum drain + normalize
            nc.vector.tensor_tensor(out=pin[:, p, :], in0=pv, in1=bc, op=MULT)

        deferred.append(emit_fin)

    # ================= emission =================
    # all q/k units first in weight-arrival order (fp8, cheap bytes), then
    # the V units - by which time the fp16 x chunk has landed
    for mt in (0, 4, 1, 5, 2, 6, 3, 7):
        p1_qk_unit(0, mt, xc0_8)
    for tt in range(4):
        p1_v_unit(0, tt, xc0)

    xc_next = {}

    def prefetch(c):
        xc_next[c] = (load_xc8(c), load_xc(c))

    def new_pin(j):
        return pin_pool.tile([128, NP, QCH], f16, name=f"pin{j}", tag="pin")

    # --- chunk 0: fillers = p1(c1) ---
    prefetch(2)
    pin0 = new_pin(0)
    fillers.extend(p1_units(1, xc1, xc1_8))
    fill_state["acc"] = 0.0
    for p in range(NP):
        attn_pair(0, p, pin0, 12.0 / 8.0)
    while fillers:
        fillers.popleft()()

    # --- chunk 1: fillers = p1 qk(c2), V(c2), qk(c3) ---
    prefetch(3)
    pin1 = new_pin(1)
    (xc2_8, xc2), (xc3_8, xc3) = xc_next[2], xc_next[3]
    fillers.extend([(lambda mt=mt: p1_qk_unit(2, mt, xc2_8)) for mt in range(8)])
    fillers.extend([(lambda tt=tt: p1_v_unit(2, tt, xc2)) for tt in range(4)])
    fillers.extend([(lambda mt=mt: p1_qk_unit(3, mt, xc3_8)) for mt in range(8)])
    fill_state["acc"] = 0.0
    for p in range(NP):
        attn_pair(1, p, pin1, 20.0 / 16.0)
    while fillers:
        fillers.popleft()()

    # --- chunks 2+3 interleaved by pair: exp-heavy chunk-3 pairs overlap
    # proj filler work instead of piling up at the end ---
    pin2 = new_pin(2)
    pin3 = new_pin(3)
    fillers.extend([(lambda tt=tt: p1_v_unit(3, tt, xc3)) for tt in range(4)])
    fillers.extend(interleave(proj_units(0, pin0), proj_units(1, pin1)))
    fill_state["acc"] = 0.0
    region = [(2, 0), (3, 0), (2, 1), (3, 1), (2, 2), (3, 2), (2, 3), (3, 3)]
    for idx, (j, p) in enumerate(region):
        if (j, p) == (3, 3):
            # drain all deferred work now: the (3,3) fillers read pin2/pin3
            # whose writers (the pending fins) must be EMITTED before them
            while deferred:
                deferred.popleft()()
            fillers.extend(interleave(
                proj_units(2, pin2),
                [(lambda mt=mt: proj3a_unit(pin3, mt)) for mt in range(8)]))
            fill_state["acc"] = -1.0 * (len(fillers) / 8.0)
            rate = len(fillers) / 8.0
        else:
            rate = 20.0 / 48.0
        attn_pair(j, p, pin3 if j == 3 else pin2, rate)
    while deferred:
        deferred.popleft()()
    while fillers:
        fillers.popleft()()
    proj3_final(pin3)


# ======================= host side =======================

_NC_CACHE = None
LAST_RESULT = None


def _get_program():
    global _NC_CACHE
    if _NC_CACHE is None:
        _NC_CACHE = build_program()
    return _NC_CACHE


def shard_inputs(x, w_qkv, b_qkv, w_proj):
    x = np.asarray(x, dtype=np.float32)
    w_qkv = np.asarray(w_qkv, dtype=np.float32)
    b_qkv = np.asarray(b_qkv, dtype=np.float32)
    w_proj = np.asarray(w_proj, dtype=np.float32)
    in_maps = []
    for c in range(NCORES):
        b = c % B
        half = c // B
        hs = half * (HPC * HD)  # 512
        wq = w_qkv[:, 0 * D + hs:0 * D + hs + HPC * HD]
        wk = w_qkv[:, 1 * D + hs:1 * D + hs + HPC * HD]
        wv_ = w_qkv[:, 2 * D + hs:2 * D + hs + HPC * HD]
        bq = b_qkv[0 * D + hs:0 * D + hs + HPC * HD]
        bk = b_qkv[1 * D + hs:1 * D + hs + HPC * HD]
        bv_ = b_qkv[2 * D + hs:2 * D + hs + HPC * HD]
        # partition-major DRAM images (8KB contiguous per partition)
        import ml_dtypes
        f8np = ml_dtypes.float8_e4m3fn
        xT = x[b].T                                          # [D, T] fp32
        xr_pm = np.ascontiguousarray(
            xT.reshape(8, 128, NCH, TCH).transpose(1, 2, 0, 3))
        wqk2 = np.concatenate([wq, wk], axis=1)              # [D, 1024] fp32
        wqk_pm = np.ascontiguousarray(
            wqk2.reshape(8, 128, 8, 128).transpose(1, 2, 0, 3)
            [:, [0, 4, 1, 5, 2, 6, 3, 7]])
        wv_r = np.ascontiguousarray(
            wv_.astype(np.float16).reshape(8, 128, HPC * HD).transpose(1, 0, 2))
        wp_r = np.ascontiguousarray(
            w_proj[hs:hs + HPC * HD, :].astype(np.float16)
            .reshape(NP, 128, D).transpose(1, 0, 2))
        bqk_r = np.ascontiguousarray(
            np.concatenate([bq, bk]).astype(np.float32).reshape(8, 128).T)
        in_maps.append({
            "xr": xr_pm.astype(np.float16),
            "xr8": xr_pm.astype(f8np),
            "wqk8": wqk_pm.astype(f8np),
            "wv": wv_r,
            "wp": wp_r,
            "bqk": bqk_r,
            "bv": np.ascontiguousarray(
                np.broadcast_to(bv_[None, :], (128, HPC * HD))).astype(np.float16),
        })
    return in_maps


def kernel(x, w_qkv, b_qkv, w_proj, b_proj):
    global LAST_RESULT
    from concourse.bass_utils import run_bass_kernel_spmd

    nc = _get_program()
    in_maps = shard_inputs(x, w_qkv, b_qkv, w_proj)
    res = run_bass_kernel_spmd(nc, in_maps, list(range(NCORES)))
    LAST_RESULT = res
    b_proj = np.asarray(b_proj, dtype=np.float32)
    y = np.empty((B, T, D), dtype=np.float32)
    for b in range(B):
        yTfull = (res.results[b]["yT"].astype(np.float32)
                  + res.results[b + B]["yT"].astype(np.float32))
        yTfull[:, 3 * QCH:4 * QCH] += (res.results[b]["yT2"].astype(np.float32)
                                       + res.results[b + B]["yT2"].astype(np.float32))
        y[b] = yTfull.T + b_proj[None, :]
    return y


# revision 69
# speedup vs baseline: 1.0089x; 1.0089x over previous
"""Causal self-attention (B=4, T=2048, D=1024, H=16, hd=64) on 8 TRN2 NeuronCores.

Sharding: core c handles batch b = c % 4 and head-half = c // 4 (8 heads each).
Each core computes, for its (batch, 8 heads):
    qkv projection -> causal attention -> partial output projection (yT).
Host gathers: y[b] = (yT[core b] + yT[core b+4]).T + b_proj.

Device design (per core), v9 (fp16 + fp8 q/k projection):
  - PSUM fp32; attention fp16/bf16; q/k projection in fp8e4m3 DoubleRow
    (2 fp8 weights per PE cell -> 256-row contraction per matmul, ~2x the
    fp16 rate; rel err 1.5e-2 vs the 2e-2 gate, deterministic)
  - transposed layouts: xT [D, T], qT/kT [2heads x 64, pair, T], V [t, kt, 8h*64]
  - S^T [k, q] tiles: two heads row-packed at tile_position (0,0)/(64,0)
  - exp on ScalarE (scale fused); causal mask via gpsimd affine_select on the
    128-wide diagonal block only
  - PV col-packed: head A -> psum rows 0:64, head B -> rows 64:128; both
    matmuls run concurrently (one 512-cyc slot)
  - softmax denominator: DVE accumulates e tiles into a per-pair partial
    (bf16 2x DVE); one col-packed pair of ones[128,64] matmuls reduces AND
    broadcasts the denominators into a [128,512] psum; DVE reciprocal -> bc;
    the PV psum drain is fused with the normalize multiply
  - attention k-tiles processed in BATCHES of 2; QK,QK back-to-back, PV
    batches deferred and popped two-at-a-time on odd batches (4 PV slots
    back-to-back).  Same-type consecutive PE slots cost ~5-30ns vs
    ~110-200ns for type changes (measured), so batching cuts most of the
    kt-loop transition overhead.
  - host pre-arranges x/wqk/wv/wp in partition-major DRAM layouts so every
    initial DMA moves 8KB-contiguous per partition; startup DMAs ride two
    queues in strict priority order (sync: xc0_8, wqk8, xc0, xc1_8, xc1, wp;
    gpsimd: bqk, bv, wv) - the aggregate HBM rate is shared at packet
    granularity, so queue-FIFO order is the only way to prioritize
  - steady-state DMA issues stay off the Scalar queue (exp cadence)
  - final projection chunk split: 3-pair partial matmuls run as filler
    inside the last attention pair and ship via yT2 (host adds), leaving
    only 8 single matmuls at the end - drained into 8 psum banks at once,
    copied out on DVE+ScalarE in parallel, shipped with 2 batched DMAs
  - emission interleaves p1/proj matmul "filler" units into the attention
    batch loop (engines are FIFO; emission order = execution order)
  - PE warmup matmuls cover the startup weight-load window and keep the HAM
    clock gate open
"""

import sys
from collections import deque
from contextlib import ExitStack

import numpy as np

sys.path.insert(0, "/opt/trn_rl_repo")

import concourse.bass as bass  # noqa: E402
import concourse.tile as tile  # noqa: E402
from concourse import bacc, mybir  # noqa: E402

f16 = mybir.dt.float16
bf16 = mybir.dt.bfloat16
f32 = mybir.dt.float32
f8 = mybir.dt.float8e4
DR = mybir.MatmulPerfMode.DoubleRow
EXP = mybir.ActivationFunctionType.Exp
ADD = mybir.AluOpType.add
MULT = mybir.AluOpType.mult

B, T, D = 4, 2048, 1024
H, HD = 16, 64
HPC = 8            # heads per core
NP = 4             # head pairs per core
NCORES = 8
TCH = 512          # p1 t-chunk width == attention q-chunk width
NCH = T // TCH     # 4
QCH = 512
NKT = T // 128     # 16 k-tiles
WARMUP = 140


def build_program():
    nc = bacc.Bacc("TRN2", target_bir_lowering=False, debug=False)

    # partition-major DRAM layouts: per-partition data is 8KB-contiguous
    xr = nc.dram_tensor("xr", [128, NCH, 8, TCH], f16, kind="ExternalInput").ap()
    # fp8 copies feeding the DoubleRow q/k projection matmuls
    xr8 = nc.dram_tensor("xr8", [128, NCH, 8, TCH], f8, kind="ExternalInput").ap()
    wqk8 = nc.dram_tensor("wqk8", [128, 8, 8, 128], f8, kind="ExternalInput").ap()
    wv = nc.dram_tensor("wv", [128, 8, HPC * HD], f16, kind="ExternalInput").ap()
    wp = nc.dram_tensor("wp", [128, NP, D], f16, kind="ExternalInput").ap()
    bqk = nc.dram_tensor("bqk", [128, 8], f32, kind="ExternalInput").ap()
    bv = nc.dram_tensor("bv", [128, HPC * HD], f16, kind="ExternalInput").ap()
    yT = nc.dram_tensor("yT", [D, T], f16, kind="ExternalOutput").ap()
    # 3-pair partial of the final projection chunk (host adds yT2 onto
    # yT[:, 1536:2048]); lets that work leave the device before the last
    # attention pair finishes
    yT2 = nc.dram_tensor("yT2", [D, QCH], f16, kind="ExternalOutput").ap()

    with tile.TileContext(nc) as tc:
        with ExitStack() as ctx:
            _build(ctx, tc, xr, xr8, wqk8, wv, wp, bqk, bv, yT, yT2)
    nc.compile()
    return nc


def _build(ctx, tc, xr, xr8, wqk8, wv, wp, bqk, bv, yT, yT2):
    nc = tc.nc

    persist = ctx.enter_context(tc.tile_pool(name="persist", bufs=1))
    w_pool = ctx.enter_context(tc.tile_pool(name="w_pool", bufs=1))
    xc_pool = ctx.enter_context(tc.tile_pool(name="xc_pool", bufs=2))
    e_pool = ctx.enter_context(tc.tile_pool(name="e_pool", bufs=14))
    pa_pool = ctx.enter_context(tc.tile_pool(name="pa_pool", bufs=3))
    pin_pool = ctx.enter_context(tc.tile_pool(name="pin_pool", bufs=4))
    bc_pool = ctx.enter_context(tc.tile_pool(name="bc_pool", bufs=2))
    ysb_pool = ctx.enter_context(tc.tile_pool(name="ysb_pool", bufs=4))
    small = ctx.enter_context(tc.tile_pool(name="small", bufs=1))

    p1ps = ctx.enter_context(tc.tile_pool(name="p1ps", bufs=2, space="PSUM"))
    s_ps = ctx.enter_context(tc.tile_pool(name="s_ps", bufs=2, space="PSUM"))
    pv_ps = ctx.enter_context(tc.tile_pool(name="pv_ps", bufs=2, space="PSUM"))

    # ---- persistent tensors ----
    qT = persist.tile([128, NP, T], f16, tag="qT")      # [2 heads x 64 dims, pair, t]
    kT = persist.tile([128, NP, T], f16, tag="kT")
    V = persist.tile([128, NKT, HPC * HD], bf16, tag="V")  # [t in tile, k-tile, h*64+d]

    ones_sb = small.tile([128, 64], bf16, tag="ones_sb", bufs=1)
    nc.vector.memset(ones_sb, 1.0)

    # ---- initial DMAs ----
    # Per-queue FIFO order = priority order.  The aggregate HBM rate is shared
    # at packet granularity across queues, so the critical first-wave items
    # (xc0, wqk mt0-1) are FIRST on their queues and nothing else competes.
    def load_xc(c, eng=None):
        xc = xc_pool.tile([128, 8, TCH], f16, name=f"xc{c}", tag="xc")
        (eng or nc.sync).dma_start(out=xc, in_=xr[:, c])
        return xc

    def load_xc8(c, eng=None):
        xc8 = xc_pool.tile([128, 8, TCH], f8, name=f"xc8_{c}", tag="xc8")
        (eng or nc.sync).dma_start(out=xc8, in_=xr8[:, c])
        return xc8

    # Two-queue strict-priority startup: everything critical goes on sync in
    # FIFO order (xc0_8 -> wqk8 -> xc0 -> xc1_8 -> xc1) so the first-wave
    # bytes are not diluted by lower-priority transfers; bulky non-critical
    # loads ride gpsimd.  Scalar issues nothing - it stays clean for exp.
    # wqk8 is stored in permuted mt order [0,4,1,5,2,6,3,7] so the first
    # 128KB slice carries exactly the q+k units of attention pair 0
    # one fast queue, strict priority: FIFO order on sync == consumption
    # order of the prologue; parallel queues only dilute the critical bytes
    xc0_8 = load_xc8(0)
    wqk_sb = w_pool.tile([128, 8, 8, 128], f8, tag="wqk_sb")
    nc.sync.dma_start(out=wqk_sb[:, 0:2], in_=wqk8[:, 0:2])
    bqk_sb = small.tile([128, 8], f32, tag="bqk_sb", bufs=1)
    nc.gpsimd.dma_start(out=bqk_sb, in_=bqk)
    nc.sync.dma_start(out=wqk_sb[:, 2:8], in_=wqk8[:, 2:8])
    bv_sb = small.tile([128, HPC * HD], f16, tag="bv_sb", bufs=1)
    nc.gpsimd.dma_start(out=bv_sb, in_=bv)
    wv_sb = w_pool.tile([128, 8, HPC * HD], f16, tag="wv_sb")
    nc.sync.dma_start(out=wv_sb, in_=wv)
    # xc0 in halves: the first v-unit's dt0-3 matmuls only need the first
    # half, which lands ~1us earlier than the whole chunk would
    xc0 = xc_pool.tile([128, 8, TCH], f16, name="xc0", tag="xc")
    nc.sync.dma_start(out=xc0[:, 0:4], in_=xr[:, 0, 0:4])
    nc.sync.dma_start(out=xc0[:, 4:8], in_=xr[:, 0, 4:8])
    xc1_8 = load_xc8(1)
    xc1 = load_xc(1)
    wp_sb = w_pool.tile([128, NP, D], f16, tag="wp_sb")
    nc.sync.dma_start(out=wp_sb, in_=wp)

    # stored slot for logical mt under the [0,4,1,5,2,6,3,7] permutation
    SLOT = {mt: (mt % 4) * 2 + (mt // 4) for mt in range(8)}

    zreg = nc.gpsimd.to_reg(0.0)

    # ---- PE warmup: keep the PE busy (and the HAM gate open) during the
    # initial weight DMA; writes a never-read scratch psum ----
    scr = p1ps.tile([128, TCH], f32, name="warm", tag="p1")
    for i in range(WARMUP):
        nc.tensor.matmul(scr[0:64, 0:64], ones_sb, ones_sb, start=True, stop=True)

    # ================= phase 1 units (qkv projection) =================
    # q/k projection in fp8 DoubleRow: 2 fp8 weights per PE cell -> each
    # matmul contracts 256 rows, halving the PE time of these units
    def p1_qk_unit(c, mt, xc8):
        qk_ps = p1ps.tile([128, TCH], f32, name=f"qk{c}_{mt}", tag="p1")
        for g in range(4):
            nc.tensor.matmul(qk_ps, wqk_sb[:, SLOT[mt], 2 * g:2 * g + 2, :],
                             xc8[:, 2 * g:2 * g + 2, :],
                             start=(g == 0), stop=(g == 3), perf_mode=DR)
        dest = qT if mt < 4 else kT
        nc.vector.tensor_scalar_add(
            dest[:, mt % 4, c * TCH:(c + 1) * TCH], qk_ps, bqk_sb[:, mt:mt + 1])

    def p1_v_unit(c, tt, xc):
        v_ps = p1ps.tile([128, 512], f32, name=f"v{c}_{tt}", tag="p1")
        for dt in range(8):
            nc.tensor.matmul(v_ps, xc[:, dt, tt * 128:(tt + 1) * 128],
                             wv_sb[:, dt, :], start=(dt == 0), stop=(dt == 7))
        nc.vector.tensor_tensor(out=V[:, c * 4 + tt, :], in0=v_ps, in1=bv_sb, op=ADD)

    def p1_units(c, xc, xc8):
        units = [(lambda c=c, mt=mt, xc8=xc8: p1_qk_unit(c, mt, xc8))
                 for mt in range(8)]
        units += [(lambda c=c, tt=tt, xc=xc: p1_v_unit(c, tt, xc)) for tt in range(4)]
        return units

    # ================= phase 3 unit (output projection) =================
    dmae_rot = [nc.sync, nc.gpsimd, nc.scalar, nc.sync]

    def proj_unit(j, pin, mt):
        y_ps = p1ps.tile([128, QCH], f32, name=f"y{j}_{mt}", tag="p1")
        for p in range(NP):
            nc.tensor.matmul(y_ps, wp_sb[:, p, mt * 128:(mt + 1) * 128],
                             pin[:, p, :], start=(p == 0), stop=(p == NP - 1))
        y_sb = ysb_pool.tile([128, QCH], f16, name=f"ysb{j}_{mt}", tag="ysb")
        if j == 3:
            # final chunk: alternate DVE/ScalarE drains; rotate DMA queues so
            # the tail does not serialize on one queue
            if mt % 2 == 0:
                nc.vector.tensor_copy(out=y_sb, in_=y_ps)
            else:
                nc.scalar.copy(out=y_sb, in_=y_ps)
            dmae_rot[mt % 4].dma_start(
                out=yT[mt * 128:(mt + 1) * 128, j * QCH:(j + 1) * QCH], in_=y_sb)
        else:
            nc.vector.tensor_copy(out=y_sb, in_=y_ps)
            nc.sync.dma_start(
                out=yT[mt * 128:(mt + 1) * 128, j * QCH:(j + 1) * QCH], in_=y_sb)

    def interleave(a, b):
        out = []
        la, lb = list(a), list(b)
        n = max(len(la), len(lb))
        for i in range(n):
            if i < len(la):
                out.append(la[i])
            if i < len(lb):
                out.append(lb[i])
        return out

    def proj_units(j, pin):
        return [(lambda j=j, pin=pin, mt=mt: proj_unit(j, pin, mt)) for mt in range(8)]

    # final chunk's projection split: a 3-pair partial that ships via yT2
    # while the last attention pair runs (host adds it), plus a 1-matmul
    # finisher per tile, so only ~2us of PE+drain work follows the last
    # attention slot
    def proj3a_unit(pin, mt):
        y_ps = p1ps.tile([128, QCH], f32, name=f"y3a_{mt}", tag="p1")
        for p in range(3):
            nc.tensor.matmul(y_ps, wp_sb[:, p, mt * 128:(mt + 1) * 128],
                             pin[:, p, :], start=(p == 0), stop=(p == 2))
        yp = ysb_pool.tile([128, QCH], f16, name=f"y3p{mt}", tag="ysb")
        nc.vector.tensor_copy(out=yp, in_=y_ps)
        [nc.sync, nc.gpsimd][mt % 2].dma_start(
            out=yT2[mt * 128:(mt + 1) * 128, :], in_=yp)

    y3b_all = persist.tile([128, 8, QCH], f16, tag="y3b")

    def proj3_final(pin):
        # by now all attention psum pools are free: run all 8 finisher
        # matmuls back-to-back into 8 banks, drain on DVE+ScalarE in
        # parallel, ship with two batched DMAs
        pss = []
        s_tiles = []
        s_full = None
        for mt in range(8):
            if mt < 2:
                ps = p1ps.tile([128, QCH], f32, name=f"y3b{mt}", tag="p1")
            elif mt < 4:
                ps = pv_ps.tile([128, QCH], f32, name=f"y3b{mt}", tag="pv")
            else:
                if mt % 2 == 0:
                    s_full = s_ps.tile([128, 2 * QCH], f32,
                                       name=f"y3s{mt}", tag="s")
                    s_tiles.append(s_full)
                ps = s_full[:, 0:QCH] if mt % 2 == 0 else s_full[:, QCH:2 * QCH]
            nc.tensor.matmul(ps, wp_sb[:, 3, mt * 128:(mt + 1) * 128],
                             pin[:, 3, :], start=True, stop=True)
            pss.append(ps)
        # alternate DVE/ScalarE drains; ship each mt-pair the moment both of
        # its drains are done (the last DMA's HBM receipt is the tail's
        # critical path, so keep it small and early)
        yv = yT.rearrange("(m p) t -> p m t", p=128)
        for mt in range(8):
            if mt % 2 == 0:
                nc.vector.tensor_copy(out=y3b_all[:, mt, :], in_=pss[mt])
            else:
                nc.scalar.copy(out=y3b_all[:, mt, :], in_=pss[mt])
                [nc.sync, nc.gpsimd][(mt // 2) % 2].dma_start(
                    out=yv[:, mt - 1:mt + 1, 3 * QCH:4 * QCH],
                    in_=y3b_all[:, mt - 1:mt + 1, :])

    # ================= phase 2: attention =================
    # deferred: PE-op closures executed with a lag of 2 kt-batches so the PE
    # (FIFO) never queues a PV matmul whose exp/mask input isn't ready yet.
    deferred = deque()
    fillers = deque()
    fill_state = {"acc": 0.0}

    def attn_pair(j, p, pin, rate):
        nkt = 4 * (j + 1)
        q0 = j * QCH
        partial = pa_pool.tile([128, 2 * QCH], bf16, name=f"pa{j}_{p}", tag="pa")
        pv = pv_ps.tile([128, QCH], f32, name=f"pv{j}_{p}", tag="pv")
        for bt in range(nkt // 2):
            kts = (2 * bt, 2 * bt + 1)
            # 1) QK slots back-to-back (same-type transitions are cheap)
            svs = []
            for kt in kts:
                o = max(0, kt - 4 * j)
                c0 = 128 * o
                s = s_ps.tile([128, 2 * QCH], f32, name=f"s{j}_{p}_{kt}", tag="s")
                nc.tensor.matmul(
                    s[:, c0:QCH], kT[0:64, p, kt * 128:(kt + 1) * 128],
                    qT[0:64, p, q0 + c0:q0 + QCH],
                    start=True, stop=True, tile_position=(0, 0))
                nc.tensor.matmul(
                    s[:, QCH + c0:2 * QCH], kT[64:128, p, kt * 128:(kt + 1) * 128],
                    qT[64:128, p, q0 + c0:q0 + QCH],
                    start=True, stop=True, tile_position=(64, 0))
                svs.append((kt, o, c0, s))
            # 2) on odd batches, pop two deferred PV batches (plus any fins)
            # so 4 PV slots run back-to-back - same-type PE transitions are
            # ~30x cheaper than type changes.  In the chunk-2/3 region keep
            # one extra batch deferred: the exp->mask->PV chain on diagonal
            # tiles was the source of recurring ~0.5us PE gaps there.
            if bt % 2 == 1:
                thr = 3 if j >= 2 else 1
                while len(deferred) > thr:
                    deferred.popleft()()
            # 3) exp + mask + denominator partials (Scalar/GpSimd/DVE)
            es = []
            for kt, o, c0, s in svs:
                e = e_pool.tile([128, 2 * QCH], bf16, name=f"e{j}_{p}_{kt}", tag="e")
                if o == 0:
                    nc.scalar.activation(e, s, EXP, scale=0.125)
                else:
                    sv = s.rearrange("p (h q) -> p h q", h=2)[:, :, c0:QCH]
                    ev = e.rearrange("p (h q) -> p h q", h=2)[:, :, c0:QCH]
                    nc.scalar.activation(ev, sv, EXP, scale=0.125)
                if kt >= 4 * j:
                    # mask only the 128-wide diagonal block: keep col >= k
                    ev2 = e.rearrange("p (h q) -> p h q", h=2)[:, :, c0:c0 + 128]
                    nc.gpsimd.affine_select(
                        ev2, ev2, pattern=[[0, 2], [1, 128]],
                        compare_op=mybir.AluOpType.is_ge, fill=zreg,
                        base=0, channel_multiplier=-1)
                if kt == 0:
                    nc.vector.tensor_copy(out=partial, in_=e)
                elif o == 0:
                    nc.vector.tensor_tensor(out=partial, in0=partial, in1=e, op=ADD)
                else:
                    pview = partial.rearrange("p (h q) -> p h q", h=2)[:, :, c0:QCH]
                    eview = e.rearrange("p (h q) -> p h q", h=2)[:, :, c0:QCH]
                    nc.vector.tensor_tensor(out=pview, in0=pview, in1=eview, op=ADD)
                es.append((kt, c0, e))

            # 4) defer this batch's PV pair-slots
            def emit_pv_batch(es=es, pv=pv, nkt=nkt, p=p):
                for kt, c0, e in es:
                    last = (kt == nkt - 1)
                    nc.tensor.matmul(
                        pv[0:64, c0:QCH], V[:, kt, (2 * p) * 64:(2 * p + 1) * 64],
                        e[:, c0:QCH], start=(kt == 0), stop=last)
                    nc.tensor.matmul(
                        pv[64:128, c0:QCH], V[:, kt, (2 * p + 1) * 64:(2 * p + 2) * 64],
                        e[:, QCH + c0:2 * QCH], start=(kt == 0), stop=last)

            deferred.append(emit_pv_batch)
            # 5) fillers - popped per batch: the steady drip spaces QK
            # batches at the exp cadence (clumping them stalls the next QK
            # batch on s_ps/exp recycling - measured +44us)
            fill_state["acc"] += rate
            while fillers and fill_state["acc"] >= 1.0:
                fillers.popleft()()
                fill_state["acc"] -= 1.0

        def emit_fin(j=j, p=p, partial=partial, pv=pv, pin=pin):
            # ones[128,64] matmuls: reduce the partial over k AND broadcast
            # the per-head denominators into all 64 rows of a psum bank
            dn = p1ps.tile([128, QCH], f32, name=f"dn{j}_{p}", tag="p1")
            nc.tensor.matmul(dn[0:64, :], ones_sb, partial[:, 0:QCH],
                             start=True, stop=True)
            nc.tensor.matmul(dn[64:128, :], ones_sb, partial[:, QCH:2 * QCH],
                             start=True, stop=True)
            bc = bc_pool.tile([128, QCH], f32, name=f"bc{j}_{p}", tag="bc")
            nc.vector.reciprocal_approx_fast(out=bc, in_=dn)
            # fused PV-psum drain + normalize
            nc.vector.tensor_tensor(out=pin[:, p, :], in0=pv, in1=bc, op=MULT)

        deferred.append(emit_fin)

    # ================= emission =================
    # all q/k units first in weight-arrival order (fp8, cheap bytes), then
    # the V units - by which time the fp16 x chunk has landed
    for mt in (0, 4, 1, 5, 2, 6, 3, 7):
        p1_qk_unit(0, mt, xc0_8)
    for tt in range(4):
        p1_v_unit(0, tt, xc0)

    xc_next = {}

    def prefetch(c):
        xc_next[c] = (load_xc8(c), load_xc(c))

    def new_pin(j):
        return pin_pool.tile([128, NP, QCH], f16, name=f"pin{j}", tag="pin")

    # --- chunk 0: fillers = p1(c1) ---
    prefetch(2)
    pin0 = new_pin(0)
    fillers.extend(p1_units(1, xc1, xc1_8))
    fill_state["acc"] = 0.0
    for p in range(NP):
        attn_pair(0, p, pin0, 12.0 / 8.0)
    while fillers:
        fillers.popleft()()

    # --- chunk 1: fillers = p1 qk(c2), V(c2), qk(c3) ---
    prefetch(3)
    pin1 = new_pin(1)
    (xc2_8, xc2), (xc3_8, xc3) = xc_next[2], xc_next[3]
    fillers.extend([(lambda mt=mt: p1_qk_unit(2, mt, xc2_8)) for mt in range(8)])
    fillers.extend([(lambda tt=tt: p1_v_unit(2, tt, xc2)) for tt in range(4)])
    fillers.extend([(lambda mt=mt: p1_qk_unit(3, mt, xc3_8)) for mt in range(8)])
    fill_state["acc"] = 0.0
    for p in range(NP):
        attn_pair(1, p, pin1, 20.0 / 16.0)
    while fillers:
        fillers.popleft()()

    # --- chunks 2+3 interleaved by pair: exp-heavy chunk-3 pairs overlap
    # proj filler work instead of piling up at the end ---
    pin2 = new_pin(2)
    pin3 = new_pin(3)
    fillers.extend([(lambda tt=tt: p1_v_unit(3, tt, xc3)) for tt in range(4)])
    fillers.extend(interleave(proj_units(0, pin0), proj_units(1, pin1)))
    fill_state["acc"] = 0.0
    region = [(2, 0), (3, 0), (2, 1), (3, 1), (2, 2), (3, 2), (2, 3), (3, 3)]
    for idx, (j, p) in enumerate(region):
        if (j, p) == (3, 3):
            # drain all deferred work now: the (3,3) fillers read pin2/pin3
            # whose writers (the pending fins) must be EMITTED before them
            while deferred:
                deferred.popleft()()
            fillers.extend(interleave(
                proj_units(2, pin2),
                [(lambda mt=mt: proj3a_unit(pin3, mt)) for mt in range(8)]))
            fill_state["acc"] = -1.0 * (len(fillers) / 8.0)
            rate = len(fillers) / 8.0
        else:
            rate = 20.0 / 48.0
        attn_pair(j, p, pin3 if j == 3 else pin2, rate)
    while deferred:
        deferred.popleft()()
    while fillers:
        fillers.popleft()()
    proj3_final(pin3)


# ======================= host side =======================

_NC_CACHE = None
LAST_RESULT = None


def _get_program():
    global _NC_CACHE
    if _NC_CACHE is None:
        _NC_CACHE = build_program()
    return _NC_CACHE


def shard_inputs(x, w_qkv, b_qkv, w_proj):
    x = np.asarray(x, dtype=np.float32)
    w_qkv = np.asarray(w_qkv, dtype=np.float32)
    b_qkv = np.asarray(b_qkv, dtype=np.float32)
    w_proj = np.asarray(w_proj, dtype=np.float32)
    in_maps = []
    for c in range(NCORES):
        b = c % B
        half = c // B
        hs = half * (HPC * HD)  # 512
        wq = w_qkv[:, 0 * D + hs:0 * D + hs + HPC * HD]
        wk = w_qkv[:, 1 * D + hs:1 * D + hs + HPC * HD]
        wv_ = w_qkv[:, 2 * D + hs:2 * D + hs + HPC * HD]
        bq = b_qkv[0 * D + hs:0 * D + hs + HPC * HD]
        bk = b_qkv[1 * D + hs:1 * D + hs + HPC * HD]
        bv_ = b_qkv[2 * D + hs:2 * D + hs + HPC * HD]
        # partition-major DRAM images (8KB contiguous per partition)
        import ml_dtypes
        f8np = ml_dtypes.float8_e4m3fn
        xT = x[b].T                                          # [D, T] fp32
        xr_pm = np.ascontiguousarray(
            xT.reshape(8, 128, NCH, TCH).transpose(1, 2, 0, 3))
        wqk2 = np.concatenate([wq, wk], axis=1)              # [D, 1024] fp32
        wqk_pm = np.ascontiguousarray(
            wqk2.reshape(8, 128, 8, 128).transpose(1, 2, 0, 3)
            [:, [0, 4, 1, 5, 2, 6, 3, 7]])
        wv_r = np.ascontiguousarray(
            wv_.astype(np.float16).reshape(8, 128, HPC * HD).transpose(1, 0, 2))
        wp_r = np.ascontiguousarray(
            w_proj[hs:hs + HPC * HD, :].astype(np.float16)
            .reshape(NP, 128, D).transpose(1, 0, 2))
        bqk_r = np.ascontiguousarray(
            np.concatenate([bq, bk]).astype(np.float32).reshape(8, 128).T)
        in_maps.append({
            "xr": xr_pm.astype(np.float16),
            "xr8": xr_pm.astype(f8np),
            "wqk8": wqk_pm.astype(f8np),
            "wv": wv_r,
            "wp": wp_r,
            "bqk": bqk_r,
            "bv": np.ascontiguousarray(
                np.broadcast_to(bv_[None, :], (128, HPC * HD))).astype(np.float16),
        })
    return in_maps


def kernel(x, w_qkv, b_qkv, w_proj, b_proj):
    global LAST_RESULT
    from concourse.bass_utils import run_bass_kernel_spmd

    nc = _get_program()
    in_maps = shard_inputs(x, w_qkv, b_qkv, w_proj)
    res = run_bass_kernel_spmd(nc, in_maps, list(range(NCORES)))
    LAST_RESULT = res
    b_proj = np.asarray(b_proj, dtype=np.float32)
    y = np.empty((B, T, D), dtype=np.float32)
    for b in range(B):
        yTfull = (res.results[b]["yT"].astype(np.float32)
                  + res.results[b + B]["yT"].astype(np.float32))
        yTfull[:, 3 * QCH:4 * QCH] += (res.results[b]["yT2"].astype(np.float32)
                                       + res.results[b + B]["yT2"].astype(np.float32))
        y[b] = yTfull.T + b_proj[None, :]
    return y
